# revision 20
# baseline (speedup 1.0000x reference)
"""DoReFa-like quantizer with per-group top-4 masking on 8 TRN2 NeuronCores.

Self-contained kernel: takes FULL inputs, shards out_c across 8 cores,
runs one SPMD Bass/Tile program (with a tiny AllReduce(max) collective
for the global max |tanh(x)|), gathers the full output.

Math notes:
  - max|tanh(x)| == tanh(max|x|)  (tanh odd+monotonic), so pass 1 is a
    pure abs-max reduce over raw x; tanh is applied once to a scalar.
  - round-to-nearest-even of s*t is computed via the magic-constant
    trick: u = fma(s, t, 1.5*2^23) rounds the mantissa at integer
    granularity; y = u - 1.5*2^23.
  - per-group top-4 with index tie-breaks: keys b_k = |y_k| + (7-k)/8
    are distinct fp16-exact values; a 27-op min/max network computes the
    4th largest per group; mask = (b_k >= t4).
"""

import sys

import numpy as np

sys.path.insert(0, "/opt/trn_rl_repo")

import concourse.bass as bass  # noqa: E402
import concourse.tile as tile  # noqa: E402
from concourse import bacc, bass_isa, library_config, mybir  # noqa: E402
from concourse.tile_rust import add_dep_helper  # noqa: E402
from concourse.bass_utils import run_bass_kernel_spmd  # noqa: E402

GROUP_SIZE = 8
KEEP = 4
C_MAGIC = 12582912.0  # 1.5 * 2**23
F32 = mybir.dt.float32
F16 = mybir.dt.float16
AF = mybir.ActivationFunctionType
ALU = mybir.AluOpType


def build_program(n_cores, o_shard, in_c, hw, bits, gc=64):
    """SPMD program for one core's shard, shaped [o_shard, in_c*hw] f32."""
    delta = float(2 ** (int(bits) - 1) - 1)
    invd = 1.0 / delta
    g = in_c // GROUP_SIZE
    row = in_c * hw
    assert in_c % GROUP_SIZE == 0 and o_shard % 128 == 0
    ot_n = o_shard // 128
    gc = min(gc, g)
    assert g % gc == 0
    ch_n = g // gc                 # chunks per o-tile
    cw = gc * GROUP_SIZE * hw      # chunk width (f32 elems)
    fw = gc * hw                   # per-k compact width (fp16 elems)
    assert fw % 2 == 0

    nc = bacc.Bacc("TRN2", target_bir_lowering=False, debug=False,
                   num_devices=n_cores)
    x_d = nc.dram_tensor("x", [o_shard, row], F32, kind="ExternalInput")
    out_d = nc.dram_tensor("out", [o_shard, row], F32, kind="ExternalOutput")

    with tile.TileContext(nc) as tc:
        with (
            tc.tile_pool(name="xio", bufs=3) as xpool,
            tc.tile_pool(name="p1", bufs=2) as p1pool,
            tc.tile_pool(name="u", bufs=1) as upool,
            tc.tile_pool(name="w16", bufs=1) as wpool,
            tc.tile_pool(name="t4p", bufs=2) as t4pool,
            tc.tile_pool(name="small", bufs=1) as spool,
            tc.tile_pool(name="dram", bufs=1, space="DRAM") as dpool,
        ):
            nc.gpsimd.load_library(library_config.mlp)

            # ---------------- phase 1: local abs-max over the shard -------
            # small chunks so the DVE reduce chain starts early; its own
            # pool so phase-2 loads (and tanh) can prefetch during the
            # collective.
            p1w = min(6144, row)
            assert row % p1w == 0
            p1n = o_shard // 128 * (row // p1w)
            lpart = spool.tile([128, p1n], F32)
            p1_last_load = None
            for ot in range(o_shard // 128):
                for c in range(row // p1w):
                    idx = ot * (row // p1w) + c
                    xt = p1pool.tile([128, p1w], F32, tag="p1x")
                    p1_last_load = nc.sync.dma_start(
                        xt[:],
                        x_d.ap()[ot * 128:(ot + 1) * 128,
                                 c * p1w:(c + 1) * p1w])
                    nc.vector.tensor_reduce(
                        lpart[:, idx:idx + 1], xt[:],
                        axis=mybir.AxisListType.X, op=ALU.max,
                        apply_absolute_value=True)
            lall = spool.tile([128, 1], F32)
            nc.vector.tensor_reduce(lall[:], lpart[:],
                                    axis=mybir.AxisListType.X, op=ALU.max)
            lred = spool.tile([128, 1], F32)
            nc.gpsimd.partition_all_reduce(lred[:], lall[:], 128,
                                           bass_isa.ReduceOp.max)

            # ------------- all-gather(local max) across cores -------------
            cc_in = dpool.tile([128, 1], F32)
            cc_out = dpool.tile([n_cores, 128], F32)
            nc.sync.dma_start(cc_in[:], lred[:])
            nc.gpsimd.collective_compute(
                "AllGather", ALU.bypass,
                replica_groups=[list(range(n_cores))],
                ins=[cc_in.opt()], outs=[cc_out.opt()])
            gx = spool.tile([128, n_cores], F32)
            nc.sync.dma_start(
                gx[:], cc_out[:].rearrange("r p -> p r"))
            gmax = spool.tile([128, 1], F32)
            nc.vector.tensor_reduce(gmax[:], gx[:],
                                    axis=mybir.AxisListType.X, op=ALU.max)

            # s = delta / tanh(gmax), per-partition scalar
            mt = spool.tile([128, 1], F32)
            nc.scalar.activation(mt[:], gmax[:], AF.Tanh)
            rm = spool.tile([128, 1], F32)
            nc.vector.reciprocal(rm[:], mt[:])
            s_t = spool.tile([128, 1], F32)
            nc.vector.tensor_scalar_mul(s_t[:], rm[:], delta)
            # -C as a per-partition bias AP (non-Copy ACT funcs need APs)
            negc = spool.tile([128, 1], F32)
            nc.gpsimd.memset(negc[:], -C_MAGIC)

            # ---------------- phase 2: quantize + top-4 mask --------------
            TT = nc.vector.tensor_tensor
            prev = None  # (xt, y, rows, cols) of the previous chunk

            def scatter_and_store(entry):
                # out = y * (1/delta), scattered back to (g,k,s) order.
                # Emitted one chunk late so this ACT op doesn't sit in
                # front of the next chunk's tanh/u in ACT program order.
                xt_, y_, rows_, cols_ = entry
                og = xt_[:].rearrange("p (g k s) -> p k g s",
                                      k=GROUP_SIZE, s=hw)
                yg = y_[:].rearrange("p (k g s) -> p k g s", g=gc, s=hw)
                nc.scalar.activation(og, yg, AF.Copy, scale=invd)
                nc.sync.dma_start(out_d.ap()[rows_, cols_], xt_[:])

            ci = 0
            for ot in range(ot_n):
                for c in range(ch_n):
                    par = ci % 2
                    ci += 1
                    rows = slice(ot * 128, (ot + 1) * 128)
                    cols = slice(c * cw, (c + 1) * cw)
                    xt = xpool.tile([128, cw], F32, tag="x")
                    ld = nc.sync.dma_start(xt[:], x_d.ap()[rows, cols])
                    # keep the DMA FIFO serving phase-1 loads first: the
                    # collective (and hence all compute) waits on them.
                    add_dep_helper(ld.ins, p1_last_load.ins, sync=False,
                                   reason="phase-1 loads first")
                    # t = tanh(x), in place
                    nc.scalar.activation(xt[:], xt[:], AF.Tanh)

                    # u_k = s*t_k + C  (magic round), gathered k-compact
                    u = upool.tile([128, cw], F32)
                    xg = xt[:].rearrange("p (g k s) -> p g k s",
                                         k=GROUP_SIZE, s=hw)
                    ug = u[:].rearrange("p (k g s) -> p k g s", g=gc, s=hw)
                    for k in range(GROUP_SIZE):
                        nc.scalar.activation(ug[:, k], xg[:, :, k, :],
                                             AF.Copy, bias=C_MAGIC,
                                             scale=s_t[:])

                    # y = u - C   (fp16; integers in [-delta, delta])
                    y = wpool.tile([128, cw], F16, tag=f"y{par}")
                    nc.vector.tensor_scalar(y[:], u[:], C_MAGIC, None,
                                            op0=ALU.subtract)
                    # b = |y| (on ACT: |u - C|), then += (7-k)/8 per k
                    # (distinct fp16-exact sort keys, index tie-break)
                    b = wpool.tile([128, cw], F16, tag=f"b{par}")
                    nc.scalar.activation(b[:], u[:], AF.Abs,
                                         bias=negc[:])
                    for k in range(GROUP_SIZE):
                        nc.vector.tensor_scalar(
                            b[:, bass.ts(k, fw)], b[:, bass.ts(k, fw)],
                            (GROUP_SIZE - 1 - k) * 0.125, None,
                            op0=ALU.add)

                    if prev is not None:
                        scatter_and_store(prev)
                        prev = None

                    tmp = wpool.tile([128, cw], F16, tag="tmp")
                    srt = wpool.tile([128, cw], F16, tag="srt")
                    ts_ = [tmp[:, bass.ts(k, fw)] for k in range(GROUP_SIZE)]
                    ss = [srt[:, bass.ts(k, fw)] for k in range(GROUP_SIZE)]

                    def pair_view(tile_, first, step, n=2):
                        # [p, n, fw] view of slices first, first+step, ...
                        return (tile_[:]
                                .rearrange("p (k f) -> p k f", k=GROUP_SIZE)
                                [:, first::step, :][:, :n, :])

                    # stage A (vectorized): hi of 4 pairs -> tmp[0..3],
                    # lo -> tmp[4..7]
                    b_even = pair_view(b, 0, 2, 4)
                    b_odd = pair_view(b, 1, 2, 4)
                    hi4 = (tmp[:, 0:4 * fw]
                           .rearrange("p (k f) -> p k f", k=4))
                    lo4 = (tmp[:, 4 * fw:8 * fw]
                           .rearrange("p (k f) -> p k f", k=4))
                    TT(hi4, b_even, b_odd, op=ALU.max)
                    TT(lo4, b_even, b_odd, op=ALU.min)
                    # stage B (vectorized over the two halves):
                    # tmp layout now [h0 h1 h2 h3 l0 l1 l2 l3];
                    # half A merges (h0,l0)x(h1,l1), half B (h2,l2)x(h3,l3)
                    # Outputs land as srt = [a1 a2 a3 a4 B4 B3 B2 B1]
                    # (B reversed) so the t4 merge pairs are unit-stride.
                    hA = pair_view(tmp, 0, 2)       # h0, h2
                    hB = pair_view(tmp, 1, 2)       # h1, h3
                    lA = pair_view(tmp, 4, 2)       # l0, l2
                    lB = pair_view(tmp, 5, 2)       # l1, l3
                    mg = wpool.tile([128, 4 * fw], F16, tag="mg")
                    mg2 = mg[:].rearrange("p (k f) -> p k f", k=4)
                    TT(pair_view(srt, 0, 7), hA, hB, op=ALU.max)  # a1|B1
                    TT(mg2[:, 0:2, :], hA, hB, op=ALU.min)        # qA|qB
                    TT(mg2[:, 2:4, :], lA, lB, op=ALU.max)        # rA|rB
                    TT(pair_view(srt, 3, 1), lA, lB, op=ALU.min)  # a4|B4
                    TT(pair_view(srt, 1, 5), mg2[:, 0:2, :],
                       mg2[:, 2:4, :], op=ALU.max)                # a2|B2
                    TT(pair_view(srt, 2, 3), mg2[:, 0:2, :],
                       mg2[:, 2:4, :], op=ALU.min)                # a3|B3

                    # srt = [a1 a2 a3 a4 B4 B3 B2 B1]
                    # t4 = max(a4, B4, min(a1,B3), min(a2,B2), min(a3,B1))
                    s3d = srt[:].rearrange("p (k f) -> p k f", k=GROUP_SIZE)
                    TT(mg2[:, 0:3, :], s3d[:, 0:3, :], s3d[:, 5:8, :],
                       op=ALU.min)                  # m1 m2 m3
                    TT(mg2[:, 3:4, :], s3d[:, 3:4, :], s3d[:, 4:5, :],
                       op=ALU.max)                  # m4 = max(a4, B4)
                    t3d = tmp[:].rearrange("p (k f) -> p k f", k=GROUP_SIZE)
                    TT(t3d[:, 0:2, :], mg2[:, 0:2, :], mg2[:, 2:4, :],
                       op=ALU.max)
                    t4 = t4pool.tile([128, fw], F16)
                    TT(t4[:], ts_[0], ts_[1], op=ALU.max)

                    # mask_k = (b_k >= t4) ; exactly 4 per group.
                    # tmp is free again -> reuse it as the mask tile.
                    t4b = (t4[:].rearrange("p (o f) -> p o f", o=1)
                           .broadcast_to([128, GROUP_SIZE, fw]))
                    b3 = b[:].rearrange("p (k f) -> p k f", k=GROUP_SIZE)
                    m3 = tmp[:].rearrange("p (k f) -> p k f", k=GROUP_SIZE)
                    TT(m3, b3, t4b, op=ALU.is_ge)
                    # y *= mask (in place)
                    TT(y[:], y[:], tmp[:], op=ALU.mult)

                    prev = (xt, y, rows, cols)
            scatter_and_store(prev)
    nc.compile()
    return nc


_CACHE = {}


def _get_program(key):
    if key not in _CACHE:
        n_cores, o_shard, in_c, hw, bits = key
        _CACHE[key] = build_program(n_cores, o_shard, in_c, hw, bits)
    return _CACHE[key]


def run(x, bits, trace=False):
    x = np.ascontiguousarray(np.asarray(x, dtype=np.float32))
    bits = int(np.asarray(bits).item())
    oc, ic, h, w = x.shape
    n_cores = 8
    o_shard = oc // n_cores
    nc = _get_program((n_cores, o_shard, ic, h * w, bits))
    xr = x.reshape(oc, ic * h * w)
    in_maps = [{"x": xr[i * o_shard:(i + 1) * o_shard]}
               for i in range(n_cores)]
    res = run_bass_kernel_spmd(nc, in_maps, list(range(n_cores)),
                               trace=trace)
    out = np.concatenate([res.results[i]["out"] for i in range(n_cores)],
                         axis=0)
    return out.reshape(oc, ic, h, w), res


def kernel(x, bits):
    out, _ = run(x, bits, trace=False)
    return out
